# revision 14
# baseline (speedup 1.0000x reference)
"""DeRA attention (2D-rope attention) Trainium2 kernel, 8-core head-parallel.

Sharding: 16 heads over 8 cores (2 heads/core). Each core computes
q/k/v projections for its 2 heads from the (replicated) input, applies
2D rope, dense attention over S=3072, and its partial output
projection o_c = attn_c @ wo_c. The host sums the 8 partials and adds bo.

Layout tricks (all host-side prep is pure indexing / casting):
 - x is cast to bf16; x^T tiles are produced on-device via DMA-transpose.
 - wq/wk columns are permuted per head to [id-evens, id-odds, rot-evens,
   rot-odds] so rope = elementwise mul with baked cos/sin tables plus a
   partition-block swap (done with 3 SBUF->SBUF DMAs).
 - scores are computed transposed (k on partitions) so the PV matmul needs
   no transpose of the probabilities; an all-ones column appended to V
   yields the softmax denominator inside the same PSUM accumulation.
"""

import sys

if "/opt/trn_rl_repo" not in sys.path:
    sys.path.insert(0, "/opt/trn_rl_repo")

import numpy as np
import ml_dtypes

BF16N = ml_dtypes.bfloat16

# Problem config (hardcoded per spec)
S = 3072
DIM = 1536
NH = 16
HD = 96
NCORES = 8
GH, GW = 48, 64
KC = DIM // 128          # 12 contraction chunks of 128
NQ = S // 512            # 6
NKC = S // 128           # 24 key chunks
QB = 1024                # q block (columns per attention round)
NQB = S // QB            # 3
SCALE = 1.0 / float(np.sqrt(HD))

# per-head column permutation: [id evens, id odds, rot evens, rot odds]
_PERM = (
    [2 * j for j in range(16)]
    + [2 * j + 1 for j in range(16)]
    + [2 * j for j in range(16, 48)]
    + [2 * j + 1 for j in range(16, 48)]
)

_STATE = {}


def _build_nc():
    import concourse.bass as bass  # noqa: F401
    import concourse.tile as tile
    from concourse import bacc, mybir
    from contextlib import ExitStack

    BF16 = mybir.dt.bfloat16
    F32 = mybir.dt.float32
    AF = mybir.ActivationFunctionType
    OP = mybir.AluOpType

    import os
    # dma_start_transpose hangs the device when several cores issue
    # transposes concurrently; default to the PE-transpose / host-xT path.
    no_dmat = os.environ.get("DERA_NO_DMAT", "1") == "1"

    nc = bacc.Bacc("TRN2", target_bir_lowering=False, debug=False,
                   num_devices=NCORES)

    if no_dmat:
        xtd = nc.dram_tensor("xT_h", [DIM, S], BF16, kind="ExternalInput")
    else:
        xd = nc.dram_tensor("x_bf", [S, DIM], BF16, kind="ExternalInput")
    wqd = nc.dram_tensor("wq_t", [128, KC * 192], BF16, kind="ExternalInput")
    wkd = nc.dram_tensor("wk_t", [128, KC * 192], BF16, kind="ExternalInput")
    wvd = nc.dram_tensor("wv_t", [128, KC * 192], BF16, kind="ExternalInput")
    wod = nc.dram_tensor("wo_t", [96, 2 * DIM], BF16, kind="ExternalInput")
    bqd = nc.dram_tensor("bq_t", [96, 2], F32, kind="ExternalInput")
    bkd = nc.dram_tensor("bk_t", [96, 2], F32, kind="ExternalInput")
    bvd = nc.dram_tensor("bv_t", [96, 2], F32, kind="ExternalInput")
    cosd = nc.dram_tensor("cos_f", [96, S], BF16, kind="ExternalInput")
    sind = nc.dram_tensor("sin_f", [96, S], BF16, kind="ExternalInput")
    outd = nc.dram_tensor("outT", [DIM, S], F32, kind="ExternalOutput")

    with tile.TileContext(nc) as tc:
        with ExitStack() as ctx:
            const = ctx.enter_context(tc.tile_pool(name="const", bufs=1))
            cos_sb = const.tile([96, S], BF16, name="cos_sb")
            nc.sync.dma_start(out=cos_sb[:], in_=cosd.ap())
            sin_sb = const.tile([96, S], BF16, name="sin_sb")
            nc.sync.dma_start(out=sin_sb[:], in_=sind.ap())
            bq_sb = const.tile([96, 2], F32, name="bq_sb")
            nc.sync.dma_start(out=bq_sb[:], in_=bqd.ap())
            bk_sb = const.tile([96, 2], F32, name="bk_sb")
            nc.sync.dma_start(out=bk_sb[:], in_=bkd.ap())
            bv_sb = const.tile([96, 2], F32, name="bv_sb")
            nc.sync.dma_start(out=bv_sb[:], in_=bvd.ap())
            wo_sb = const.tile([96, 2 * DIM], BF16, name="wo_sb")
            nc.sync.dma_start(out=wo_sb[:], in_=wod.ap())
            if no_dmat:
                from concourse.masks import make_identity
                ident = const.tile([96, 96], BF16, name="ident")
                make_identity(nc, ident[:])

            # tensors that persist from projection phase into attention
            mid = ctx.enter_context(tc.tile_pool(name="mid", bufs=1))
            v1 = {}
            attn = {}
            rope_out = {}
            for h in (0, 1):
                v1[h] = mid.tile([128, NKC * 128], BF16, name=f"v1_{h}",
                                 tag=f"v1_{h}")
                attn[h] = mid.tile([96, S], BF16, name=f"attn_{h}",
                                   tag=f"attn_{h}")
                for t in ("q", "k"):
                    rope_out[(t, h)] = mid.tile([96, S], BF16,
                                                name=f"r_{t}{h}",
                                                tag=f"r_{t}{h}")

            # ---- phase 1: projections + rope + v layout ----
            with tc.tile_pool(name="p3", bufs=1) as p3, \
                 tc.tile_pool(name="p3ps", bufs=8, space="PSUM") as p3ps:
                xt = []
                for kc in range(KC):
                    t_ = p3.tile([128, S], BF16, name=f"xt{kc}", tag="xt",
                                 bufs=KC)
                    if no_dmat:
                        nc.sync.dma_start(
                            out=t_[:],
                            in_=xtd.ap()[kc * 128:(kc + 1) * 128, :])
                    else:
                        nc.sync.dma_start_transpose(
                            out=t_[:], in_=xd.ap()[:, kc * 128:(kc + 1) * 128])
                    xt.append(t_)
                wq_sb = p3.tile([128, KC * 192], BF16, name="wq_sb")
                nc.sync.dma_start(out=wq_sb[:], in_=wqd.ap())
                wk_sb = p3.tile([128, KC * 192], BF16, name="wk_sb")
                nc.sync.dma_start(out=wk_sb[:], in_=wkd.ap())
                wv_sb = p3.tile([128, KC * 192], BF16, name="wv_sb")
                nc.sync.dma_start(out=wv_sb[:], in_=wvd.ap())

                def project(wsb, bsb, h, dest):
                    ps = [p3ps.tile([96, 512], F32, name=f"ps{n}", tag="proj",
                                    bufs=6)
                          for n in range(NQ)]
                    for kc in range(KC):
                        lhs = wsb[:, kc * 192 + h * 96: kc * 192 + (h + 1) * 96]
                        for n in range(NQ):
                            nc.tensor.matmul(
                                ps[n][:], lhs,
                                xt[kc][:, n * 512:(n + 1) * 512],
                                start=(kc == 0), stop=(kc == KC - 1))
                    for n in range(NQ):
                        nc.scalar.activation(
                            dest[:, n * 512:(n + 1) * 512], ps[n][:],
                            AF.Identity, bias=bsb[:, h:h + 1])

                for h in (0, 1):
                    for (t, wsb, bsb) in (("q", wq_sb, bq_sb),
                                          ("k", wk_sb, bk_sb)):
                        pre = p3.tile([96, S], BF16, name=f"pre_{t}{h}",
                                      tag="pre", bufs=2)
                        project(wsb, bsb, h, pre)
                        # rope: dst = pre*cosF + swap(pre)*sinF
                        sw = p3.tile([96, S], BF16, name=f"sw_{t}{h}",
                                     tag="sw", bufs=1)
                        nc.sync.dma_start(out=sw[0:32, :], in_=pre[0:32, :])
                        nc.sync.dma_start(out=sw[32:64, :], in_=pre[64:96, :])
                        nc.sync.dma_start(out=sw[64:96, :], in_=pre[32:64, :])
                        t1 = p3.tile([96, S], BF16, name=f"t1_{t}{h}",
                                     tag="t1", bufs=1)
                        nc.vector.tensor_tensor(t1[:], pre[:], cos_sb[:],
                                                OP.mult)
                        t2 = p3.tile([96, S], BF16, name=f"t2_{t}{h}",
                                     tag="t2", bufs=1)
                        nc.vector.tensor_tensor(t2[:], sw[:], sin_sb[:],
                                                OP.mult)
                        nc.vector.tensor_tensor(rope_out[(t, h)][:], t1[:],
                                                t2[:], OP.add)
                    vt_h = p3.tile([96, S], BF16, name=f"vt{h}", tag="vt",
                                   bufs=1)
                    project(wv_sb, bv_sb, h, vt_h)
                    if no_dmat:
                        for kc2 in range(NKC):
                            tr = p3ps.tile([128, 96], BF16,
                                           name=f"tr{h}_{kc2}", tag="tr",
                                           bufs=2)
                            nc.tensor.transpose(
                                tr[:], vt_h[:, kc2 * 128:(kc2 + 1) * 128],
                                ident[:])
                            nc.scalar.activation(
                                v1[h][:, kc2 * 128: kc2 * 128 + 96], tr[:],
                                AF.Copy)
                    else:
                        for kc2 in range(NKC):
                            nc.sync.dma_start_transpose(
                                out=v1[h][:, kc2 * 128: kc2 * 128 + 96],
                                in_=vt_h[:, kc2 * 128:(kc2 + 1) * 128])
                    ones_ap = v1[h].rearrange("p (k c) -> p k c", c=128)
                    nc.gpsimd.memset(ones_ap[:, :, 96:97], 1.0)

            # ---- phase 2: attention ----
            import os
            n_rounds = int(os.environ.get("DERA_ROUNDS", "6"))
            do_p6 = os.environ.get("DERA_P6", "1") == "1"
            rounds = [(h, qb) for h in (0, 1) for qb in range(NQB)][:n_rounds]
            with tc.tile_pool(name="p5", bufs=1) as p5, \
                 tc.tile_pool(name="p5ps", bufs=2, space="PSUM") as ps_s, \
                 tc.tile_pool(name="pvps", bufs=2, space="PSUM") as ps_pv:
                for (h, qb) in rounds:
                    if True:
                        qT = rope_out[("q", h)]
                        kT = rope_out[("k", h)]
                        q0 = qb * QB
                        pv0 = ps_pv.tile([97, 512], F32, name=f"pv0_{h}{qb}",
                                         tag="pv0", bufs=2)
                        pv1 = ps_pv.tile([97, 512], F32, name=f"pv1_{h}{qb}",
                                         tag="pv1", bufs=2)
                        for kc2 in range(NKC):
                            sps = ps_s.tile([128, 1024], F32,
                                            name=f"s_{h}{qb}_{kc2}", tag="s",
                                            bufs=2)
                            lhs_k = kT[:, kc2 * 128:(kc2 + 1) * 128]
                            nc.tensor.matmul(sps[:, 0:512], lhs_k,
                                             qT[:, q0: q0 + 512])
                            nc.tensor.matmul(sps[:, 512:1024], lhs_k,
                                             qT[:, q0 + 512: q0 + 1024])
                            pt = p5.tile([128, 1024], BF16,
                                         name=f"pt_{h}{qb}_{kc2}", tag="pt",
                                         bufs=3)
                            nc.scalar.activation(pt[:], sps[:], AF.Exp,
                                                 scale=SCALE)
                            lhs_v = v1[h][:, kc2 * 128: kc2 * 128 + 97]
                            nc.tensor.matmul(pv0[:], lhs_v, pt[:, 0:512],
                                             start=(kc2 == 0),
                                             stop=(kc2 == NKC - 1))
                            nc.tensor.matmul(pv1[:], lhs_v, pt[:, 512:1024],
                                             start=(kc2 == 0),
                                             stop=(kc2 == NKC - 1))
                        for j, pv in enumerate((pv0, pv1)):
                            den = p5.tile([97, 512], F32, name=f"den{h}{qb}{j}",
                                          tag="den", bufs=2)
                            nc.vector.tensor_copy(den[96:97, :], pv[96:97, :])
                            dn0 = p5.tile([1, 512], F32, name=f"dn0{h}{qb}{j}",
                                          tag="dn0", bufs=2)
                            nc.sync.dma_start(out=dn0[:], in_=den[96:97, :])
                            rec = p5.tile([1, 512], F32, name=f"rec{h}{qb}{j}",
                                          tag="rec", bufs=2)
                            nc.vector.reciprocal(rec[:], dn0[:])
                            rb = p5.tile([96, 512], F32, name=f"rb{h}{qb}{j}",
                                         tag="rb", bufs=2)
                            nc.gpsimd.partition_broadcast(rb[:], rec[:])
                            nc.vector.tensor_tensor(
                                attn[h][:, q0 + j * 512: q0 + (j + 1) * 512],
                                pv[0:96, :], rb[:], OP.mult)

            # ---- phase 3: output projection ----
            with tc.tile_pool(name="p6", bufs=1) as p6, \
                 tc.tile_pool(name="p6ps", bufs=4, space="PSUM") as p6ps:
                for oc in range(KC if do_p6 else 0):
                    for n in range(NQ):
                        po = p6ps.tile([128, 512], F32, name=f"po{oc}_{n}",
                                       tag="po")
                        nc.tensor.matmul(
                            po[:], wo_sb[:, oc * 128:(oc + 1) * 128],
                            attn[0][:, n * 512:(n + 1) * 512],
                            start=True, stop=False)
                        nc.tensor.matmul(
                            po[:], wo_sb[:, DIM + oc * 128: DIM + (oc + 1) * 128],
                            attn[1][:, n * 512:(n + 1) * 512],
                            start=False, stop=True)
                        ot = p6.tile([128, 512], F32, name=f"ot{oc}_{n}",
                                     tag="ot", bufs=6)
                        if n % 2:
                            nc.scalar.activation(ot[:], po[:], AF.Copy)
                        else:
                            nc.vector.tensor_copy(ot[:], po[:])
                        nc.sync.dma_start(
                            out=outd.ap()[oc * 128:(oc + 1) * 128,
                                          n * 512:(n + 1) * 512],
                            in_=ot[:])

    nc.compile()
    return nc


def _get_nc():
    if "nc" not in _STATE:
        _STATE["nc"] = _build_nc()
    return _STATE["nc"]


def make_in_maps(x, wq, bq, wk, bk, wv, bv, wo, bo, freqs_cos, freqs_sin,
                 h, w):
    """Host-side shard prep: pure indexing/casting, returns per-core in_maps."""
    assert int(h) == GH and int(w) == GW
    x = np.asarray(x, np.float32)
    wq = np.asarray(wq, np.float32)
    wk = np.asarray(wk, np.float32)
    wv = np.asarray(wv, np.float32)
    wo = np.asarray(wo, np.float32)
    bq = np.asarray(bq, np.float32)
    bk = np.asarray(bk, np.float32)
    bv = np.asarray(bv, np.float32)
    fc = np.asarray(freqs_cos, np.float32)
    fs = np.asarray(freqs_sin, np.float32)

    perm = np.asarray(_PERM)

    # rope tables in the permuted row basis
    tpos = np.arange(S)
    gh = tpos // GW
    gw = tpos % GW
    c32 = np.empty((32, S), np.float32)
    s32 = np.empty((32, S), np.float32)
    c32[0:16] = fc[gh, 16:32].T
    c32[16:32] = fc[gw, 32:48].T
    s32[0:16] = fs[gh, 16:32].T
    s32[16:32] = fs[gw, 32:48].T
    cosF = np.ones((96, S), np.float32)
    cosF[32:64] = c32
    cosF[64:96] = c32
    sinF = np.zeros((96, S), np.float32)
    sinF[32:64] = -s32
    sinF[64:96] = s32
    cosF = cosF.astype(BF16N)
    sinF = sinF.astype(BF16N)

    x_bf = np.ascontiguousarray(x[0]).astype(BF16N)
    xT_h = np.ascontiguousarray(x_bf.T)

    def tile_w(wc):
        # [1536, 192] -> [128, KC*192] with col block kc = rows kc*128..+128
        return np.ascontiguousarray(
            wc.reshape(KC, 128, 192).transpose(1, 0, 2).reshape(128, KC * 192)
        ).astype(BF16N)

    in_maps = []
    for c in range(NCORES):
        h0, h1 = 2 * c, 2 * c + 1
        qk_cols = np.concatenate([h0 * HD + perm, h1 * HD + perm])
        v_cols = np.arange(h0 * HD, (h1 + 1) * HD)
        wq_c = tile_w(wq[:, qk_cols])
        wk_c = tile_w(wk[:, qk_cols])
        wv_c = tile_w(wv[:, v_cols])
        woc = wo[h0 * HD:(h1 + 1) * HD, :]       # [192, 1536]
        wo_c = np.ascontiguousarray(
            woc.reshape(2, 96, DIM).transpose(1, 0, 2).reshape(96, 2 * DIM)
        ).astype(BF16N)
        bq_c = np.stack([bq[h0 * HD + perm], bq[h1 * HD + perm]], axis=1)
        bk_c = np.stack([bk[h0 * HD + perm], bk[h1 * HD + perm]], axis=1)
        bv_c = np.stack([bv[v_cols[:HD]], bv[v_cols[HD:]]], axis=1)
        in_maps.append({
            "x_bf": x_bf, "xT_h": xT_h,
            "wq_t": wq_c, "wk_t": wk_c, "wv_t": wv_c, "wo_t": wo_c,
            "bq_t": np.ascontiguousarray(bq_c, dtype=np.float32),
            "bk_t": np.ascontiguousarray(bk_c, dtype=np.float32),
            "bv_t": np.ascontiguousarray(bv_c, dtype=np.float32),
            "cos_f": cosF, "sin_f": sinF,
        })
    return in_maps


def assemble_output(partials, bo):
    """partials: list of 8 [DIM, S] fp32 arrays (o^T per core)."""
    acc = np.zeros((DIM, S), np.float64)
    for p in partials:
        acc += p.astype(np.float64)
    out = acc.T + np.asarray(bo, np.float64)[None, :]
    return out[None].astype(np.float32)


def kernel(x, wq, bq, wk, bk, wv, bv, wo, bo, freqs_cos, freqs_sin, h, w):
    from concourse.bass_utils import run_bass_kernel_spmd

    nc = _get_nc()
    in_maps = make_in_maps(x, wq, bq, wk, bk, wv, bv, wo, bo,
                           freqs_cos, freqs_sin, h, w)
    res = run_bass_kernel_spmd(nc, in_maps, core_ids=list(range(NCORES)))
    partials = [res.results[c]["outT"] for c in range(NCORES)]
    return assemble_output(partials, bo)
